# revision 3
# baseline (speedup 1.0000x reference)
"""Trainium2 Bass kernel for the attention module:

    s = einsum('bqd,bad->bqa', q, a)      # [B, Nq, Na]
    e = softmax(s, axis=1)                 # over the Nq axis
    e = e / sum(e, axis=1)                 # identity (col sums are 1)
    h = einsum('bqa,bqd->bad', e, q)       # [B, Na, D]

Strategy: pure data parallel over B across 8 NeuronCores (2 batches/core).
Per batch, loop over j-tiles (128 rows of the output / columns of s):
  gemm1: sT[j, i] = aT.T @ qT     (contraction over d, f32r full-speed PE)
  softmax along the free axis of the [128, Nq] PSUM block
  PE-transpose e back to [i, j] tiles for gemm2's stationary operand
  gemm2: h[j, d] = e.T @ q_nat    (contraction over i)
  scale rows by 1/rowsum, DMA out.

All matmul operands are float32r (TF32-like, 1 cycle/row, ~1.5e-4 rel err).
"""

import numpy as np

import concourse.bass as bass
import concourse.tile as tile
from concourse import bacc, mybir
from concourse.masks import make_identity

f32 = mybir.dt.float32
f32r = mybir.dt.float32r
AX = mybir.AxisListType
ALU = mybir.AluOpType
ACTF = mybir.ActivationFunctionType

P = 128

B, NQ, NA, D = 16, 2048, 2048, 1024
NCORES = 8
BLOC = B // NCORES


def build(bloc=BLOC, nq=NQ, na=NA, d=D, reps=1, num_devices=NCORES):
    """Build the per-core Bass program. All sizes must be multiples of 128."""
    ni = nq // P            # i-tiles (q rows)
    nj = na // P            # j-tiles (a rows / output rows)
    nd = d // P             # d-tiles (contraction of gemm1)
    s_q = min(512, nq)      # gemm1 moving strip (over i)
    s_d = min(512, d)       # gemm2 moving strip (over d)
    nstrip_q = nq // s_q
    nstrip_d = d // s_d

    nc = bacc.Bacc("TRN2", target_bir_lowering=False, debug=False,
                   num_devices=num_devices)
    q_d = nc.dram_tensor("q", [bloc, nq, d], f32r, kind="ExternalInput").ap()
    a_d = nc.dram_tensor("a", [bloc, na, d], f32r, kind="ExternalInput").ap()
    h_d = nc.dram_tensor("h", [bloc, na, d], f32, kind="ExternalOutput").ap()

    from contextlib import ExitStack

    with tile.TileContext(nc) as tc, ExitStack() as ctx:
        const = ctx.enter_context(tc.tile_pool(name="const", bufs=1))
        qpool = ctx.enter_context(tc.tile_pool(name="qpool", bufs=1))
        apool = ctx.enter_context(tc.tile_pool(name="apool", bufs=2))
        epool = ctx.enter_context(tc.tile_pool(name="epool", bufs=2))
        espool = ctx.enter_context(tc.tile_pool(name="espool", bufs=1))
        hpool = ctx.enter_context(tc.tile_pool(name="hpool", bufs=2))
        stat = ctx.enter_context(tc.tile_pool(name="stat", bufs=3))
        ps_s = ctx.enter_context(tc.tile_pool(name="ps_s", bufs=1, space="PSUM"))
        ps_h = ctx.enter_context(tc.tile_pool(name="ps_h", bufs=1, space="PSUM"))
        ps_tr = ctx.enter_context(tc.tile_pool(name="ps_tr", bufs=2, space="PSUM"))

        if True:
            id32 = const.tile([P, P], f32)
            make_identity(nc, id32)
            idr = const.tile([P, P], f32r)
            nc.vector.tensor_copy(idr[:], id32[:])

            def body():
                for b in range(bloc):
                    emit_batch(b)

            def emit_batch(b):
                # ---- q prologue: load q natural, build qT by PE transpose
                q_nat = qpool.tile([P, ni, d], f32r, name="q_nat")
                for ik in range(ni):
                    nc.sync.dma_start(out=q_nat[:, ik, :],
                                      in_=q_d[b, ik * P:(ik + 1) * P, :])
                qT = qpool.tile([P, nd, nq], f32r, name="qT")
                qT_v = qT.rearrange("p nd (ni i) -> p nd ni i", ni=ni)
                for ik in range(ni):
                    for g in range(0, nd, 4):
                        gw = min(4, nd - g)
                        ptr_q = ps_tr.tile([P, 4, P], f32r, name="ptr", tag="ptr")
                        for m in range(gw):
                            nc.tensor.transpose(
                                ptr_q[:, m, :],
                                q_nat[:, ik, (g + m) * P:(g + m + 1) * P],
                                idr[:])
                        nc.vector.tensor_copy(qT_v[:, g:g + gw, ik, :],
                                              ptr_q[:, 0:gw, :])

                def a_prep(jt):
                    a_nat = apool.tile([P, d], f32r, name="a_nat")
                    nc.sync.dma_start(out=a_nat[:],
                                      in_=a_d[b, jt * P:(jt + 1) * P, :])
                    aT = apool.tile([P, nd, P], f32r, name="aT")
                    for g in range(0, nd, 4):
                        gw = min(4, nd - g)
                        ptr_a = ps_tr.tile([P, 4, P], f32r, name="ptr", tag="ptr")
                        for m in range(gw):
                            nc.tensor.transpose(
                                ptr_a[:, m, :],
                                a_nat[:, (g + m) * P:(g + m + 1) * P],
                                idr[:])
                        nc.vector.tensor_copy(aT[:, g:g + gw, :],
                                              ptr_a[:, 0:gw, :])
                    return aT

                def gemm1(aT, psum_sT):
                    for k in range(nd):
                        for st in range(nstrip_q):
                            nc.tensor.matmul(
                                psum_sT[:, st * s_q:(st + 1) * s_q],
                                aT[:, k, :],
                                qT[:, k, st * s_q:(st + 1) * s_q],
                                start=(k == 0), stop=(k == nd - 1))

                def stats(psum_sT):
                    nm = stat.tile([P, 1], f32, name="nm")
                    nc.vector.tensor_reduce(nm[:], psum_sT[:], axis=AX.X,
                                            op=ALU.max, negate=True)
                    eT = epool.tile([P, nq], f32r, name="eT")
                    S = stat.tile([P, 1], f32, name="S")
                    nc.scalar.activation(eT[:], psum_sT[:], ACTF.Exp,
                                         bias=nm[:], scale=1.0, accum_out=S[:])
                    rS = stat.tile([P, 1], f32, name="rS")
                    nc.vector.reciprocal(rS[:], S[:])
                    return eT, rS

                def consume(jt, eT, rS):
                    # e-transposes: eT [j, i] -> e_sb [i-part, ik, j]
                    e_sb = espool.tile([P, ni, P], f32r, name="e_sb")
                    for g in range(0, ni, 4):
                        gw = min(4, ni - g)
                        ptr_e = ps_tr.tile([P, 4, P], f32r, name="ptr", tag="ptr")
                        for m in range(gw):
                            nc.tensor.transpose(
                                ptr_e[:, m, :],
                                eT[:, (g + m) * P:(g + m + 1) * P],
                                idr[:])
                        nc.scalar.copy(e_sb[:, g:g + gw, :], ptr_e[:, 0:gw, :])
                    psum_h = ps_h.tile([P, d], f32, name="psum_h")
                    for ik in range(ni):
                        for st in range(nstrip_d):
                            nc.tensor.matmul(
                                psum_h[:, st * s_d:(st + 1) * s_d],
                                e_sb[:, ik, :],
                                q_nat[:, ik, st * s_d:(st + 1) * s_d],
                                start=(ik == 0), stop=(ik == ni - 1))
                    h_sb = hpool.tile([P, d], f32, name="h_sb")
                    nc.vector.tensor_scalar_mul(h_sb[:], psum_h[:], rS[:])
                    nc.sync.dma_start(out=h_d[b, jt * P:(jt + 1) * P, :],
                                      in_=h_sb[:])

                # ---- software-pipelined j-tile loop
                aT = a_prep(0)
                pending = None
                for jt in range(nj):
                    psum_sT = ps_s.tile([P, nq], f32, name="psum_sT")
                    gemm1(aT, psum_sT)
                    eT, rS = stats(psum_sT)
                    if pending is not None:
                        consume(*pending)
                    if jt + 1 < nj:
                        aT = a_prep(jt + 1)
                    pending = (jt, eT, rS)
                consume(*pending)

            if reps == 1:
                body()
            else:
                with tc.For_i(0, reps, 1):
                    body()

    nc.compile()
    return nc


_CACHE = {}


def _get_program():
    key = "main"
    if key not in _CACHE:
        _CACHE[key] = build()
    return _CACHE[key]


def kernel(q: np.ndarray, a: np.ndarray) -> np.ndarray:
    from concourse import bass_utils

    q = np.ascontiguousarray(np.asarray(q, dtype=np.float32))
    a = np.ascontiguousarray(np.asarray(a, dtype=np.float32))
    assert q.shape == (B, NQ, D) and a.shape == (B, NA, D), (q.shape, a.shape)

    nc = _get_program()
    in_maps = []
    for c in range(NCORES):
        lo, hi = c * BLOC, (c + 1) * BLOC
        in_maps.append({"q": q[lo:hi], "a": a[lo:hi]})
    res = bass_utils.run_bass_kernel_spmd(nc, in_maps, core_ids=list(range(NCORES)))
    out = np.concatenate([res.results[c]["h"] for c in range(NCORES)], axis=0)
    return out
